# revision 3
# baseline (speedup 1.0000x reference)
"""V4: routed kernel, all-bf16, DMA-count-minimized + engine-balanced.

Data-parallel over 8 cores (1024 rows each), weights replicated bf16.
Host sorts each core's rows by expert per module type; groups padded to
C=288.  Key changes vs V3:

- One weight DMA per (layer, expert): host pre-rearranges W{j}1 to
  [M, 128, Kc*H] so each expert's stack loads as a single 0.5-1MB DMA
  (112 -> 16 weight DMAs; ~60us less DMA-issue serialization).
- No K=1 bias matmuls on PE: swapped-layer bias is added at PSUM
  evacuation via tensor_tensor with a host-replicated [128, H] bias
  tile; relu is deferred past the permute (elementwise relu commutes
  with token permutation) and applied to the transposed zx tiles.
- Scatters batched 12 -> 2 per boundary (full chunks in one indirect
  DMA with a [128, 8] offset table, tails in a second).
- PSUM evacuations rotate across Act/DVE/Pool (tensor_scalar dual-op
  add+max does bias+relu on the vector engines).
- First/big layers all bf16 (incl. xe), halving const DMA bytes.
- Big-layer k-accumulation runs h-part first so the x-part transpose
  latency of the previous boundary hides behind real PE work.
"""

import numpy as np
import ml_dtypes
from contextlib import ExitStack

import concourse.bass as bass
import concourse.bacc as bacc
import concourse.tile as tile
import concourse.mybir as mybir
from concourse import bass_utils

F32 = mybir.dt.float32
BF16 = mybir.dt.bfloat16
I32 = mybir.dt.int32
RELU = mybir.ActivationFunctionType.Relu
COPY = mybir.ActivationFunctionType.Copy
ADD = mybir.AluOpType.add
MAX = mybir.AluOpType.max

B = 8192
NCORES = 8
BC = B // NCORES
FEAT = 32
M = 4
H = 512
OUT = 8
P = 128
C0 = 288                     # default group capacity (multiple of 32)
KBIG = [4, 8, 8, 8]          # k-chunks (128 rows each) per big layer


def _geom(C):
    """Chunk geometry: full 128-row chunks first (slot-major), tails last."""
    nf = C // P              # full chunks per expert
    rt = C - nf * P          # tail rows per expert (may be 0)
    chunks = []              # (slot, m, g0, r)
    for m in range(M):
        for fi in range(nf):
            chunks.append((m * nf + fi, m, C * m + fi * P, P))
    if rt:
        for m in range(M):
            chunks.append((M * nf + m, m, C * m + nf * P, rt))
    nch = len(chunks)
    return chunks, nf, rt, nch


def _emit(nc, tc, ctx, d, C):
    Bp = M * C
    chunks, nf, rt, NCH = _geom(C)
    NFULL = M * nf

    consts = ctx.enter_context(tc.tile_pool(name="consts", bufs=1))
    wpool = ctx.enter_context(tc.tile_pool(name="wbig", bufs=3))
    hp = ctx.enter_context(tc.tile_pool(name="hacts", bufs=3))
    zp = ctx.enter_context(tc.tile_pool(name="zperm", bufs=2))
    xp = ctx.enter_context(tc.tile_pool(name="xsc", bufs=2))
    outp = ctx.enter_context(tc.tile_pool(name="outs", bufs=1))
    psA = ctx.enter_context(tc.tile_pool(name="psumA", bufs=4, space="PSUM"))
    psB = ctx.enter_context(tc.tile_pool(name="psumB", bufs=3, space="PSUM"))

    # ---------------- constants (7 DMAs) ----------------
    xe = consts.tile([P, 4, Bp], BF16, tag="xe", name="xe")
    nc.sync.dma_start(xe[:], d["xe"].ap().rearrange("j p b -> p j b"))
    wf = consts.tile([P, 4, H], BF16, tag="wf", name="wf")
    nc.sync.dma_start(wf[:], d["Wf"].ap().rearrange("j p h -> p j h"))
    w32 = consts.tile([P, M, 4, OUT], BF16, tag="w32", name="w32")
    nc.sync.dma_start(w32[:], d["W32"].ap())
    bias_sb = consts.tile([P, 8 * 16], F32, tag="bias", name="bias")
    nc.sync.dma_start(bias_sb[:], d["bias"].ap())
    btile = consts.tile([P, 3 * M, H], BF16, tag="btile", name="btile")
    nc.gpsimd.dma_start(btile[:], d["btile"].ap())
    bh = consts.tile([OUT, 4], F32, tag="bh", name="bh")
    nc.sync.dma_start(bh[:], d["bh"].ap())
    tbl = consts.tile([P, 3 * NCH], I32, tag="tbl", name="tbl")
    nc.sync.dma_start(tbl[:], d["tbl"].ap())

    def bias_ap(layer, hh, m):
        col = layer * 16 + hh * 4 + m
        return bias_sb[:, col:col + 1]

    # GPSIMD cannot touch PSUM: ACT takes feature-major bias+relu evacs,
    # DVE takes batch-major bias-row adds; gpsimd gets SBUF-only work + SWDGE.
    def evac_biasrelu(out, ps, bap):
        nc.scalar.activation(out, ps, RELU, bias=bap)

    def evac_addbias(out, ps, brow_ap):
        nc.vector.tensor_tensor(out, ps, brow_ap, ADD)

    # ---------------- layers ----------------
    def first_layer(j):
        """h[:, hh, m*C:..] = relu(Wf[j].T @ xe[j] + b_j0), feature-major."""
        h = hp.tile([P, 4, Bp], BF16, tag="h", name=f"h{j}")
        for hh in range(4):
            for m in range(M):
                ps = psA.tile([P, C], F32, tag="pa", name="pa")
                nc.tensor.matmul(ps[:], wf[:, j, bass.ts(hh, P)],
                                 xe[:, j, bass.ts(m, C)], start=True, stop=True)
                evac_biasrelu(h[:, hh, bass.ts(m, C)], ps[:], bias_ap(2 * j, hh, m))
        return h

    def load_w(j):
        ws = []
        for m in range(M):
            w = wpool.tile([P, KBIG[j] * H], BF16, tag="wt", name=f"w{j}{m}")
            nc.gpsimd.dma_start(w[:], d[f"Wb{j}"].ap()[m])
            ws.append(w)
        return ws

    def zslice(zpair, h, j, k, m, g0, r):
        """k-th 128-row slice of this stage's concat input, cols g0:g0+r."""
        if j == 0:
            return h[:, k, g0:g0 + r]
        if k >= 4:
            return h[:, k - 4, g0:g0 + r]
        zt = zpair[m // 2]
        off = g0 - (m // 2) * 2 * C
        return zt[:, k, off:off + r]

    def swapped_big(j, zpair, h):
        """batch-major psum per chunk; evac adds bias row; no relu here."""
        Kc = KBIG[j]
        xsc = xp.tile([P, NCH, H], BF16, tag="xsc", name=f"xsc{j}")
        horder = list(range(4, Kc)) + list(range(0, min(4, Kc)))  # h-part first
        if j == 0:
            horder = list(range(Kc))
        for m in range(M):
            mch = [c for c in chunks if c[1] == m]
            pbs = {}
            for (slot, _, g0, r) in mch:
                pb = psB.tile([P, H], F32, tag="pb", name="pb")
                pbs[slot] = pb
                for ki, k in enumerate(horder[:Kc // 2]):
                    nc.tensor.matmul(pb[:r, :], zslice(zpair, h, j, k, m, g0, r),
                                     wsj[m][:, bass.ts(k, H)],
                                     start=(ki == 0), stop=False)
            for (slot, _, g0, r) in mch:
                pb = pbs[slot]
                for ki, k in enumerate(horder[Kc // 2:]):
                    nc.tensor.matmul(pb[:r, :], zslice(zpair, h, j, k, m, g0, r),
                                     wsj[m][:, bass.ts(k, H)],
                                     start=False, stop=(ki == Kc // 2 - 1))
                evac_addbias(xsc[:r, slot, :], pb[:r, :], btile[:r, j * M + m, :])
        return xsc

    def transition(t_i, xsc):
        """Scatter tokens into next stage's order; XBAR back feature-major."""
        xb = d["xb"][t_i]
        for (slot, m, g0, r) in chunks:
            nc.gpsimd.indirect_dma_start(
                xb.ap(),
                bass.IndirectOffsetOnAxis(
                    ap=tbl[:r, t_i * NCH + slot:t_i * NCH + slot + 1], axis=0),
                xsc[:r, slot, :], None)
        zpair = []
        for half in range(2):
            zt = zp.tile([P, 4, 2 * C], BF16, tag=f"z{half}", name=f"z{half}")
            for k in range(4):
                nc.sync.dma_start(
                    zt[:, k, :],
                    xb.ap()[half * 2 * C:(half + 1) * 2 * C,
                            bass.ts(k, P)], transpose=True)
            # relu in place (SBUF-only), split across engines for latency
            nc.gpsimd.tensor_scalar(zt[:, 0:2, :], zt[:, 0:2, :], 0.0, None, MAX)
            nc.vector.tensor_scalar(zt[:, 2:4, :], zt[:, 2:4, :], 0.0, None, MAX)
            zpair.append(zt)
        return zpair

    def grouped_big(j, zpair, h):
        """relu(W_j1[m].T @ z + b): feature-major grouped output."""
        Kc = KBIG[j]
        x4 = hp.tile([P, 4, Bp], BF16, tag="h", name="x4")
        for m in range(M):
            ps4 = [psA.tile([P, C], F32, tag="pa", name="pa") for _ in range(4)]
            korder = list(range(4, Kc)) + list(range(4))
            for ki, k in enumerate(korder):
                z = zslice(zpair, h, j, k, m, m * C, C)
                for hh in range(4):
                    nc.tensor.matmul(
                        ps4[hh][:], wsj[m][:, k * H + hh * P:k * H + (hh + 1) * P],
                        z, start=(ki == 0), stop=(ki == Kc - 1))
            for hh in range(4):
                evac_biasrelu(x4[:, hh, bass.ts(m, C)], ps4[hh][:],
                              bias_ap(7, hh, m))
        return x4

    # ---------------- network ----------------
    h0 = first_layer(0)
    h1 = first_layer(1)
    wsj = load_w(0)
    xsc = swapped_big(0, None, h0)
    zpair = transition(0, xsc)
    wsj = load_w(1)
    h2 = first_layer(2)
    xsc = swapped_big(1, zpair, h1)
    zpair = transition(1, xsc)
    wsj = load_w(2)
    h3 = first_layer(3)
    xsc = swapped_big(2, zpair, h2)
    zpair = transition(2, xsc)
    wsj = load_w(3)
    x4 = grouped_big(3, zpair, h3)

    # head
    out_t = outp.tile([OUT, Bp], F32, tag="outt", name="outt")
    for m in range(M):
        ph = psB.tile([OUT, C], F32, tag="pb", name="ph")
        for k in range(4):
            nc.tensor.matmul(ph[:], w32[:, m, k, :], x4[:, k, bass.ts(m, C)],
                             start=(k == 0), stop=(k == 3))
        nc.vector.tensor_scalar_add(
            out_t[:, bass.ts(m, C)], ph[:], bh[:, m:m + 1])
    nc.sync.dma_start(d["out"].ap(), out_t[:])


def build_program(C=C0, reps: int = 1):
    Bp = M * C
    _, _, _, NCH = _geom(C)
    nc = bacc.Bacc("TRN2", target_bir_lowering=False, debug=False,
                   enable_asserts=False)
    d = {}
    d["xe"] = nc.dram_tensor("xe", [4, P, Bp], BF16, kind="ExternalInput")
    d["Wf"] = nc.dram_tensor("Wf", [4, P, H], BF16, kind="ExternalInput")
    for j in range(4):
        d[f"Wb{j}"] = nc.dram_tensor(f"Wb{j}", [M, P, KBIG[j] * H], BF16,
                                     kind="ExternalInput")
    d["W32"] = nc.dram_tensor("W32", [P, M, 4, OUT], BF16, kind="ExternalInput")
    d["bias"] = nc.dram_tensor("bias", [P, 8 * 16], F32, kind="ExternalInput")
    d["btile"] = nc.dram_tensor("btile", [P, 3 * M * H], BF16,
                                kind="ExternalInput")
    d["bh"] = nc.dram_tensor("bh", [OUT, 4], F32, kind="ExternalInput")
    d["tbl"] = nc.dram_tensor("tbl", [P, 3 * NCH], I32, kind="ExternalInput")
    d["out"] = nc.dram_tensor("out", [OUT, Bp], F32, kind="ExternalOutput")
    d["xb"] = [nc.dram_tensor(f"xb{i}", [Bp, H], BF16, kind="Internal")
               for i in range(3)]

    with tile.TileContext(nc) as tc, ExitStack() as ctx:
        if reps == 1:
            _emit(nc, tc, ctx, d, C)
        else:
            with tc.For_i(0, reps, 1):
                _emit(nc, tc, ctx, d, C)
    nc.compile()
    return nc


def prep_inputs(inputs):
    iv = np.asarray(inputs["input_val"], dtype=np.float32)
    feats = iv[:, :4 * FEAT]
    oh = iv[:, 4 * FEAT:4 * FEAT + 16]
    idx = [np.argmax(oh[:, 4 * j:4 * j + 4], axis=1) for j in range(4)]

    Cmax = 0
    for c in range(NCORES):
        rc = slice(c * BC, (c + 1) * BC)
        for j in range(4):
            Cmax = max(Cmax, int(np.bincount(idx[j][rc], minlength=M).max()))
    C = max(C0, ((Cmax + 31) // 32) * 32)
    Bp = M * C
    chunks, nf, rt, NCH = _geom(C)

    tobf = lambda a: np.ascontiguousarray(
        np.asarray(a, np.float32).astype(ml_dtypes.bfloat16))

    bias = np.zeros((P, 8 * 16), np.float32)
    for j in range(4):
        bl = np.asarray(inputs[f"b{j}_0"], np.float32)
        for hh in range(4):
            for m in range(M):
                bias[:, 2 * j * 16 + hh * 4 + m] = bl[m, hh * P:(hh + 1) * P]
    b31 = np.asarray(inputs["b3_1"], np.float32)
    for hh in range(4):
        for m in range(M):
            bias[:, 7 * 16 + hh * 4 + m] = b31[m, hh * P:(hh + 1) * P]

    btile = np.zeros((P, 3 * M * H), np.float32)
    for t, nm in enumerate(("b0_1", "b1_1", "b2_1")):
        bl = np.asarray(inputs[nm], np.float32)
        for m in range(M):
            btile[:, (t * M + m) * H:(t * M + m + 1) * H] = bl[m][None, :]
    bh = np.ascontiguousarray(np.asarray(inputs["b3_2"], np.float32).T)

    Wf = np.stack([np.asarray(inputs[f"W{j}_0"], np.float32).reshape(P, H)
                   for j in range(4)])
    Wb = {}
    for j in range(4):
        Kc = KBIG[j]
        W = np.asarray(inputs[f"W{j}_1"], np.float32)        # [M, Kc*128, H]
        Wb[j] = tobf(W.reshape(M, Kc, P, H).transpose(0, 2, 1, 3)
                     .reshape(M, P, Kc * H))
    W32 = tobf(np.asarray(inputs["W3_2"], np.float32)
               .reshape(M, 4, P, OUT).transpose(2, 0, 1, 3))   # [P, M, 4, OUT]

    shared = {
        "Wf": tobf(Wf), "bias": bias, "bh": bh, "btile": tobf(btile),
        "W32": W32,
        **{f"Wb{j}": Wb[j] for j in range(4)},
    }

    in_maps, meta = [], []
    for c in range(NCORES):
        rc = slice(c * BC, (c + 1) * BC)
        orders, slots, padlists = [], [], []
        for j in range(4):
            ij = idx[j][rc]
            order = np.full(Bp, -1, np.int64)
            slot = np.empty(BC, np.int64)
            pads = []
            for m in range(M):
                rows = np.nonzero(ij == m)[0]
                order[C * m:C * m + len(rows)] = rows
                slot[rows] = C * m + np.arange(len(rows))
                pads.extend(range(C * m + len(rows), C * (m + 1)))
            orders.append(order)
            slots.append(slot)
            padlists.append(np.array(pads, np.int64))

        xe = np.zeros((4, P, Bp), np.float32)
        for j in range(4):
            ij = idx[j][rc]
            fj = feats[rc, FEAT * j:FEAT * (j + 1)]
            for m in range(M):
                rows = np.nonzero(ij == m)[0]
                xe[j, m * FEAT:(m + 1) * FEAT, C * m:C * m + len(rows)] = \
                    fj[rows].T

        tblv = np.zeros((P, 3 * NCH), np.int32)
        for t in range(3):
            jp, jn = t, t + 1
            padmap = {int(g): i for i, g in enumerate(padlists[jp])}
            for (slot_i, m, g0, r) in chunks:
                for p in range(r):
                    g = g0 + p
                    s = orders[jp][g]
                    if s >= 0:
                        tblv[p, t * NCH + slot_i] = slots[jn][s]
                    else:
                        tblv[p, t * NCH + slot_i] = padlists[jn][padmap[g]]
        in_maps.append({"xe": tobf(xe), "tbl": tblv, **shared})
        meta.append(slots[3])
    return C, in_maps, meta


_CACHE = {}


def kernel(**inputs):
    C, in_maps, meta = prep_inputs(inputs)
    if ("nc", C) not in _CACHE:
        _CACHE[("nc", C)] = build_program(C)
    nc = _CACHE[("nc", C)]
    res = bass_utils.run_bass_kernel_spmd(
        nc, in_maps, core_ids=list(range(NCORES)))
    out = np.empty((B, OUT), np.float32)
    for c in range(NCORES):
        o = res.results[c]["out"]
        out[c * BC:(c + 1) * BC] = o[:, meta[c]].T
    return out


if __name__ == "__main__":
    import sys, jax
    import reference
    cpu = jax.local_devices(backend="cpu")[0]
    with jax.default_device(cpu):
        inputs = {k: np.asarray(v) for k, v in reference.setup_inputs().items()}
        exp = np.asarray(reference.reference(**inputs))
    if len(sys.argv) > 1 and sys.argv[1] == "sim":
        from concourse.bass_interp import CoreSim
        C, in_maps, meta = prep_inputs(inputs)
        nc = build_program(C)
        sim = CoreSim(nc, trace=len(sys.argv) > 2)
        for k, v in in_maps[0].items():
            sim.tensor(k)[:] = v
        sim.simulate()
        print(f"sim trace_time: {sim.trace_time} ns")
        o = np.asarray(sim.tensor("out"))
        got0 = o[:, meta[0]].T
        exp0 = exp[:BC]
        err = np.abs(got0 - exp0)
        print(f"sim core0 max abs err: {err.max():.3e}  "
              f"rel: {err.max()/np.abs(exp0).max():.3e}")
    else:
        got = kernel(**inputs)
        err = np.abs(got - exp)
        print(f"max abs err: {err.max():.3e}   "
              f"rel: {err.max()/np.abs(exp).max():.3e}")
